# revision 1
# baseline (speedup 1.0000x reference)
"""JPEG compression roundtrip kernel for Trainium2 (8 NeuronCores, batch-parallel).

Self-contained: builds constants, shards batch 32 -> 8 cores x 4 images,
runs a Bass/Tile kernel per core, gathers full output.

Pipeline per image (512x512x3 f32 in [0,1)):
  S1  u8 = floor(255*x) via rne(255*x - 0.5)           [ACT + DVE]
  p1  (stationary=u8 chunks, moving=color-scaled DCT)  -> M1 = (A@{Y,Cb,Cr})^T
  p2  (stationary=DCT const, moving=M1)                -> coef' [fw, fh]
  q   deq = rne(coef*1/t)*t                            [DVE, DVE, GPSIMD]
  p3  (stationary=deq chunks, moving=IDCT const)       -> M3 [fh, w]
  p4  (stationary=IDCT+color consts, moving=M3)        -> R,G,B planes in PSUM
  S5  out = min(max(rne(v),0),255)/255, interleave     [DVE, GPSIMD, ACT]

The 4:2:0 chroma down/upsample is folded into the chroma DCT matrices
(E = D@P, V = 2E^T); the +-128 level shifts cancel exactly because the DC
quant step (2) divides the DC shift (1024).
"""
import numpy as np

from concourse import bacc, bass, mybir, tile
from concourse.bass_utils import run_bass_kernel_spmd

F = np.float32
C_RNE = float(np.float32(12582912.0))  # 1.5 * 2**23
N_CORES = 8
B_PER_CORE = 4
DT = mybir.dt.float32
DT_MM = mybir.dt.float32
DT_BF = mybir.dt.bfloat16
QUALITY = 95

_LUMA = np.array([
    [16, 11, 10, 16, 24, 40, 51, 61],
    [12, 12, 14, 19, 26, 58, 60, 55],
    [14, 13, 16, 24, 40, 57, 69, 56],
    [14, 17, 22, 29, 51, 87, 80, 62],
    [18, 22, 37, 56, 68, 109, 103, 77],
    [24, 35, 55, 64, 81, 104, 113, 92],
    [49, 64, 78, 87, 103, 121, 120, 101],
    [72, 92, 95, 98, 112, 100, 103, 99]], dtype=F)
_CHROMA = np.array([
    [17, 18, 24, 47, 99, 99, 99, 99],
    [18, 21, 26, 66, 99, 99, 99, 99],
    [24, 26, 56, 99, 99, 99, 99, 99],
    [47, 66, 99, 99, 99, 99, 99, 99],
    [99, 99, 99, 99, 99, 99, 99, 99],
    [99, 99, 99, 99, 99, 99, 99, 99],
    [99, 99, 99, 99, 99, 99, 99, 99],
    [99, 99, 99, 99, 99, 99, 99, 99]], dtype=F)


def _qtable(base, quality):
    scale = 5000.0 / quality if quality < 50 else 200.0 - 2.0 * quality
    return np.clip(np.floor((base * scale + 50.0) / 100.0), 1.0, 255.0).astype(F)


def build_consts():
    k = np.arange(8)
    D = np.sqrt(2.0 / 8.0) * np.cos((2 * k[None, :] + 1) * k[:, None] * np.pi / 16.0)
    D[0, :] /= np.sqrt(2.0)
    D = D.astype(F)
    P = np.zeros((8, 16), F)
    for i in range(8):
        P[i, 2 * i] = 0.5
        P[i, 2 * i + 1] = 0.5
    E = (D @ P).astype(F)
    V = (2.0 * E.T).astype(F)
    QL = _qtable(_LUMA, QUALITY)
    QC = _qtable(_CHROMA, QUALITY)
    I16 = np.eye(16, dtype=F)
    I8 = np.eye(8, dtype=F)
    cY = np.array([0.299, 0.587, 0.114], F)
    cCb = np.array([-0.168736, -0.331264, 0.5], F)
    cCr = np.array([0.5, -0.418688, -0.081312], F)

    c = {}
    mv_fy = np.kron(I16, D.T).astype(F)
    mv_fc = np.kron(I8, E.T).astype(F)
    import ml_dtypes
    for ch in range(3):
        mv = np.ascontiguousarray(np.concatenate(
            [cY[ch] * mv_fy, cCb[ch] * mv_fc, cCr[ch] * mv_fc], axis=1).astype(F))
        hi = mv.astype(ml_dtypes.bfloat16)
        lo = (mv - hi.astype(F)).astype(ml_dtypes.bfloat16)
        c[f"mvp1_{ch}_hi"] = hi
        c[f"mvp1_{ch}_lo"] = lo
    c["sp2y"] = mv_fy.copy()
    c["sp2c"] = np.ascontiguousarray(np.pad(mv_fc, ((0, 0), (0, 64))))
    c["mvp3y"] = np.kron(I16, D).astype(F)
    c["mvp3c"] = np.kron(I16, V.T).astype(F)
    def _bfsplit(name, m):
        hi = m.astype(ml_dtypes.bfloat16)
        c[name + "_hi"] = hi
        c[name + "_lo"] = (m - hi.astype(F)).astype(ml_dtypes.bfloat16)
    _bfsplit("sp4y", np.kron(I16, D).astype(F))
    sp4c = np.kron(I16, V).T.astype(F)  # [128 fhc, 256 h]
    wR_cr, wG_cb, wG_cr, wB_cb = 1.402, -0.344136, -0.714136, 1.772
    for h in range(2):
        sl = np.ascontiguousarray(sp4c[:, 128 * h:128 * (h + 1)])
        _bfsplit(f"sp4c_h{h}_rcr", (F(wR_cr) * sl).astype(F))
        _bfsplit(f"sp4c_h{h}_gcb", (F(wG_cb) * sl).astype(F))
        _bfsplit(f"sp4c_h{h}_gcr", (F(wG_cr) * sl).astype(F))
        _bfsplit(f"sp4c_h{h}_bcb", (F(wB_cb) * sl).astype(F))
    tY = np.empty((128, 512), F)
    pp, ff = np.meshgrid(np.arange(128), np.arange(512), indexing="ij")
    tY[:] = QL[ff % 8, pp % 8]
    tC = np.empty((128, 256), F)
    pp, ff = np.meshgrid(np.arange(128), np.arange(256), indexing="ij")
    tC[:] = QC[ff % 8, pp % 8]
    c["taby"] = tY.reshape(128, 4, 128).copy()
    c["rtaby"] = (1.0 / tY).astype(F).reshape(128, 4, 128).copy()
    c["tabc"] = tC.reshape(128, 2, 128).copy()
    c["rtabc"] = (1.0 / tC).astype(F).reshape(128, 2, 128).copy()
    return c


BF_CONSTS = {"mvp1_0_hi", "mvp1_1_hi", "mvp1_2_hi", "mvp1_0_lo", "mvp1_1_lo", "mvp1_2_lo", "sp4y_hi", "sp4c_h0_rcr_hi", "sp4c_h0_gcb_hi", "sp4c_h0_gcr_hi", "sp4c_h0_bcb_hi", "sp4c_h1_rcr_hi", "sp4c_h1_gcb_hi", "sp4c_h1_gcr_hi", "sp4c_h1_bcb_hi", "sp4y_lo", "sp4c_h0_rcr_lo", "sp4c_h0_gcb_lo", "sp4c_h0_gcr_lo", "sp4c_h0_bcb_lo", "sp4c_h1_rcr_lo", "sp4c_h1_gcb_lo", "sp4c_h1_gcr_lo", "sp4c_h1_bcb_lo"}
MM_CONSTS = {"sp2y", "sp2c", "mvp3y", "mvp3c"}

CONST_SHAPES = {
    "mvp1_0_hi": (128, 256), "mvp1_0_lo": (128, 256),
    "mvp1_1_hi": (128, 256), "mvp1_1_lo": (128, 256),
    "mvp1_2_hi": (128, 256), "mvp1_2_lo": (128, 256),
    "sp2y": (128, 128), "sp2c": (128, 128),
    "mvp3y": (128, 128), "mvp3c": (128, 256),
    "sp4y_hi": (128, 128), "sp4y_lo": (128, 128),
    "sp4c_h0_rcr_hi": (128, 128), "sp4c_h0_rcr_lo": (128, 128),
    "sp4c_h0_gcb_hi": (128, 128), "sp4c_h0_gcb_lo": (128, 128),
    "sp4c_h0_gcr_hi": (128, 128), "sp4c_h0_gcr_lo": (128, 128),
    "sp4c_h0_bcb_hi": (128, 128), "sp4c_h0_bcb_lo": (128, 128),
    "sp4c_h1_rcr_hi": (128, 128), "sp4c_h1_rcr_lo": (128, 128),
    "sp4c_h1_gcb_hi": (128, 128), "sp4c_h1_gcb_lo": (128, 128),
    "sp4c_h1_gcr_hi": (128, 128), "sp4c_h1_gcr_lo": (128, 128),
    "sp4c_h1_bcb_hi": (128, 128), "sp4c_h1_bcb_lo": (128, 128),
    "taby": (128, 4, 128), "rtaby": (128, 4, 128),
    "tabc": (128, 2, 128), "rtabc": (128, 2, 128),
}


def _mm_ap(ap):
    return ap


def build_nc():
    Alu = mybir.AluOpType
    Act = mybir.ActivationFunctionType
    nc = bacc.Bacc("TRN2", target_bir_lowering=False, debug=False,
                   num_devices=N_CORES)
    x_d = nc.dram_tensor("x", [B_PER_CORE, 512, 512, 3], DT,
                         kind="ExternalInput").ap()
    o_d = nc.dram_tensor("out", [B_PER_CORE, 512, 512, 3], DT,
                         kind="ExternalOutput").ap()
    def _cdt(k):
        return DT_BF if k in BF_CONSTS else DT
    cd = {k: nc.dram_tensor(k, list(s), _cdt(k), kind="ExternalInput").ap()
          for k, s in CONST_SHAPES.items()}

    with tile.TileContext(nc) as tc:
        with (
            tc.tile_pool(name="cpool", bufs=1) as cpool,
            tc.tile_pool(name="iopool", bufs=3) as iopool,
            tc.tile_pool(name="u8pool", bufs=5) as u8pool,
            tc.tile_pool(name="m1pool", bufs=5) as m1pool,
            tc.tile_pool(name="m2pool", bufs=5) as m2pool,
            tc.tile_pool(name="m3pool", bufs=5) as m3pool,
            tc.tile_pool(name="ppool", bufs=4) as ppool,
            tc.tile_pool(name="pspool", bufs=6, space="PSUM") as pspool,
        ):
            ct = {}
            for k, s in CONST_SHAPES.items():
                ct[k] = cpool.tile(list(s), _cdt(k), tag=k, name=k)
                nc.sync.dma_start(out=ct[k][:], in_=cd[k][:])

            for b in range(B_PER_CORE):
                # ---- S1: load + floor(255*x) ----
                u8 = []
                for r in range(4):
                    xin = iopool.tile([128, 512, 3], DT, tag="xin", name="xin")
                    nc.sync.dma_start(out=xin[:], in_=x_d[b, 128 * r:128 * (r + 1)])
                    u8t = u8pool.tile([128, 512, 3], DT_BF, tag="u8", name="u8t")
                    nc.scalar.activation(xin[:], xin[:], Act.Copy,
                                         bias=-0.5, scale=255.0)
                    nc.vector.tensor_scalar(
                        out=u8t[:], in0=xin[:], scalar1=C_RNE, scalar2=C_RNE,
                        op0=Alu.add, op1=Alu.subtract)
                    u8.append(u8t)

                # ---- p1: M1 = (A @ plane)^T for Y/Cb/Cr at once ----
                m1y, m1cb, m1cr = [], [], []
                for jc in range(4):
                    psA = pspool.tile([128, 2, 256], DT, tag="ps", name="psA")
                    psB = pspool.tile([128, 2, 256], DT, tag="ps", name="psB")
                    for r in range(4):
                        pst = psA if r < 2 else psB
                        g = r % 2
                        idx = 0
                        for ch in range(3):
                            stat = u8[r][:, 128 * jc:128 * (jc + 1), ch]
                            for part in ("hi", "lo"):
                                nc.tensor.matmul(
                                    pst[:, g, :], stat,
                                    ct[f"mvp1_{ch}_{part}"][:],
                                    start=(idx == 0), stop=(idx == 5))
                                idx += 1
                    yt = m1pool.tile([128, 4, 128], DT_MM, tag="m1y", name="yt")
                    cbt = m1pool.tile([128, 4, 64], DT_MM, tag="m1cb", name="cbt")
                    crt = m1pool.tile([128, 4, 64], DT_MM, tag="m1cr", name="crt")
                    nc.scalar.copy(yt[:, 0:2, :], psA[:, :, 0:128])
                    nc.scalar.copy(yt[:, 2:4, :], psB[:, :, 0:128])
                    nc.vector.tensor_copy(cbt[:, 0:2, :], psA[:, :, 128:192])
                    nc.vector.tensor_copy(cbt[:, 2:4, :], psB[:, :, 128:192])
                    nc.vector.tensor_copy(crt[:, 0:2, :], psA[:, :, 192:256])
                    nc.vector.tensor_copy(crt[:, 2:4, :], psB[:, :, 192:256])
                    m1y.append(yt)
                    m1cb.append(cbt)
                    m1cr.append(crt)

                # ---- p2 + quant: luma ----
                m2qy = []
                for r2 in range(4):
                    ps2 = pspool.tile([128, 4, 128], DT, tag="ps", name="ps2")
                    nc.tensor.matmul(ps2[:], _mm_ap(ct["sp2y"][:]),
                                     _mm_ap(m1y[r2][:]), start=True, stop=True)
                    qt = m2pool.tile([128, 4, 128], DT_MM, tag="m2qy", name="qty")
                    nc.vector.tensor_tensor(
                        out=qt[:], in0=ps2[:], in1=ct["rtaby"][:], op=Alu.mult)
                    nc.vector.tensor_scalar(
                        out=qt[:], in0=qt[:], scalar1=C_RNE, scalar2=C_RNE,
                        op0=Alu.add, op1=Alu.subtract)
                    nc.gpsimd.tensor_tensor(
                        out=qt[:], in0=qt[:], in1=ct["taby"][:], op=Alu.mult)
                    m2qy.append(qt)

                # ---- p2 + quant: chroma (pairs of 64-row outputs) ----
                m2qc = {0: [], 1: []}
                for chi, m1c in ((0, m1cb), (1, m1cr)):
                    for t_ in range(2):
                        qt = m2pool.tile([128, 2, 128], DT_MM, tag="m2qc", name="qtc")
                        for half in range(2):
                            r2 = 2 * t_ + half
                            psc = pspool.tile([128, 2, 128], DT, tag="ps",
                                              name="psc")
                            nc.tensor.matmul(
                                psc[:], _mm_ap(ct["sp2c"][:]),
                                _mm_ap(m1c[r2][:]), start=True, stop=True)
                            nc.vector.tensor_tensor(
                                out=qt[64 * half:64 * (half + 1), :, :],
                                in0=psc[0:64, :, :], in1=ct["rtabc"][0:64, :, :],
                                op=Alu.mult)
                        nc.vector.tensor_scalar(
                            out=qt[:], in0=qt[:], scalar1=C_RNE, scalar2=C_RNE,
                            op0=Alu.add, op1=Alu.subtract)
                        nc.gpsimd.tensor_tensor(
                            out=qt[:], in0=qt[:], in1=ct["tabc"][:], op=Alu.mult)
                        m2qc[chi].append(qt)

                # ---- p3: luma -> M3 [fh, w] ----
                m3y = []
                for jc3 in range(4):
                    ps3 = pspool.tile([128, 4, 128], DT, tag="ps", name="ps3")
                    for r3 in range(4):
                        nc.tensor.matmul(
                            ps3[:, r3, :], _mm_ap(m2qy[r3][:, jc3, :]),
                            _mm_ap(ct["mvp3y"][:]), start=True, stop=True)
                    mt = m3pool.tile([128, 4, 128], DT_BF, tag="m3y", name="mty")
                    mtl = m3pool.tile([128, 4, 128], DT_BF, tag="m3yl", name="mtyl")
                    nc.scalar.copy(mt[:], ps3[:])
                    nc.vector.tensor_tensor(out=mtl[:], in0=ps3[:], in1=mt[:],
                                            op=Alu.subtract)
                    m3y.append((mt, mtl))

                # ---- p3: chroma -> M3c [fhc, w] ----
                m3c = {0: [], 1: []}
                for chi in (0, 1):
                    for jc3 in range(2):
                        ps3 = pspool.tile([128, 2, 256], DT, tag="ps", name="psA")
                        for r3 in range(2):
                            nc.tensor.matmul(
                                ps3[:, r3, :], _mm_ap(m2qc[chi][r3][:, jc3, :]),
                                _mm_ap(ct["mvp3c"][:]), start=True, stop=True)
                        mt = m3pool.tile([128, 2, 256], DT_BF, tag="m3c", name="mtc")
                        mtl = m3pool.tile([128, 2, 256], DT_BF, tag="m3cl", name="mtcl")
                        nc.scalar.copy(mt[:], ps3[:])
                        nc.vector.tensor_tensor(out=mtl[:], in0=ps3[:], in1=mt[:],
                                                op=Alu.subtract)
                        m3c[chi].append((mt, mtl))

                # ---- p4 + color + post + store ----
                for r in range(4):
                    rc, half = divmod(r, 2)
                    psR = pspool.tile([128, 512], DT, tag="ps", name="psR")
                    psG = pspool.tile([128, 512], DT, tag="ps", name="psG")
                    psB4 = pspool.tile([128, 512], DT, tag="ps", name="psB4")
                    my = m3y[r]
                    mcb = m3c[0][rc]
                    mcr = m3c[1][rc]

                    def _acc(ps, terms):
                        mms = []
                        for cname, (mh, ml) in terms:
                            sh = ct[cname + "_hi"][:]
                            sl = ct[cname + "_lo"][:]
                            mms += [(sh, mh[:]), (sh, ml[:]), (sl, mh[:])]
                        for i, (a_, b_) in enumerate(mms):
                            nc.tensor.matmul(ps[:], a_, b_, start=(i == 0),
                                             stop=(i == len(mms) - 1))
                    _acc(psR, [("sp4y", my), (f"sp4c_h{half}_rcr", mcr)])
                    _acc(psG, [("sp4y", my), (f"sp4c_h{half}_gcb", mcb),
                               (f"sp4c_h{half}_gcr", mcr)])
                    _acc(psB4, [("sp4y", my), (f"sp4c_h{half}_bcb", mcb)])
                    ot = iopool.tile([128, 512, 3], DT, tag="o", name="ot")
                    for chn, ps in ((0, psR), (1, psG), (2, psB4)):
                        pt = ppool.tile([128, 512], DT, tag="post", name="pt")
                        nc.vector.tensor_scalar(
                            out=pt[:], in0=ps[:], scalar1=C_RNE, scalar2=C_RNE,
                            op0=Alu.add, op1=Alu.subtract)
                        nc.gpsimd.tensor_scalar(
                            out=pt[:], in0=pt[:], scalar1=255.0, scalar2=0.0,
                            op0=Alu.min, op1=Alu.max)
                        nc.scalar.activation(ot[:, :, chn], pt[:], Act.Copy,
                                             bias=0.0, scale=float(F(1.0) / F(255.0)))
                    nc.sync.dma_start(out=o_d[b, 128 * r:128 * (r + 1)], in_=ot[:])

    nc.compile()
    return nc


_CACHE = {}


def kernel(x: np.ndarray) -> np.ndarray:
    assert x.shape == (32, 512, 512, 3)
    if "nc" not in _CACHE:
        _CACHE["nc"] = build_nc()
        _CACHE["consts"] = build_consts()
    nc = _CACHE["nc"]
    consts = _CACHE["consts"]
    xs = np.ascontiguousarray(x.astype(F))
    in_maps = []
    for i in range(N_CORES):
        m = {"x": xs[B_PER_CORE * i:B_PER_CORE * (i + 1)]}
        m.update(consts)
        in_maps.append(m)
    res = run_bass_kernel_spmd(nc, in_maps, list(range(N_CORES)))
    out = np.concatenate([res.results[i]["out"] for i in range(N_CORES)], axis=0)
    return out.astype(np.float32)



# revision 19
# speedup vs baseline: 1.9581x; 1.9581x over previous
"""JPEG compression roundtrip kernel for Trainium2 (8 NeuronCores, batch-parallel).

Self-contained: builds constants, shards batch 32 -> 8 cores x 4 images,
runs a Bass/Tile kernel per core, gathers full output.

All-fp16 matmul pipeline (fp32 PSUM accumulation). u8 pixels (<=255) and
dequantized coefficients (integers <=2047) are exact in fp16; DCT constants
are fp16-rounded (11-bit) keeping total rel err ~4e-3 vs the fp32 reference.
fp16 operands cost 1 PE cycle/row (vs 4 for fp32).

Pipeline per image (512x512x3 f32 in [0,1)):
  S1a  u8 = floor(255*x) via uint8-convert of 255x-0.5 (convert rounds rne)
  S1b  u8 -> fp16 (exact)
  p1   vDCT + color fold -> m1[jc] [128w, 4band, 256(128 Yfh|64 cb|64 cr)]
  p2   hDCT luma/chroma (chroma h-downsample folded into E)
  q    k=rne(coef*rtab); deq=k*tab (fp16 exact ints)
  p3   hIDCT (+ chroma h-upsample via V)
  p4   vIDCT + color, consts pre-scaled by 1/255
  post clip(v,0,1) -> interleaved out tile
The +-128 level shifts cancel exactly (DC quant step 2 divides the shifts).
"""
import numpy as np
import ml_dtypes

from concourse import bacc, bass, mybir, tile
from concourse.bass_utils import run_bass_kernel_spmd

F = np.float32
C_RNE = float(np.float32(12582912.0))  # 1.5 * 2**23
N_CORES = 8
B_PER_CORE = 4
DT = mybir.dt.float32
F16 = mybir.dt.float16
U8 = mybir.dt.uint8
QUALITY = 95

_LUMA = np.array([
    [16, 11, 10, 16, 24, 40, 51, 61],
    [12, 12, 14, 19, 26, 58, 60, 55],
    [14, 13, 16, 24, 40, 57, 69, 56],
    [14, 17, 22, 29, 51, 87, 80, 62],
    [18, 22, 37, 56, 68, 109, 103, 77],
    [24, 35, 55, 64, 81, 104, 113, 92],
    [49, 64, 78, 87, 103, 121, 120, 101],
    [72, 92, 95, 98, 112, 100, 103, 99]], dtype=F)
_CHROMA = np.array([
    [17, 18, 24, 47, 99, 99, 99, 99],
    [18, 21, 26, 66, 99, 99, 99, 99],
    [24, 26, 56, 99, 99, 99, 99, 99],
    [47, 66, 99, 99, 99, 99, 99, 99],
    [99, 99, 99, 99, 99, 99, 99, 99],
    [99, 99, 99, 99, 99, 99, 99, 99],
    [99, 99, 99, 99, 99, 99, 99, 99],
    [99, 99, 99, 99, 99, 99, 99, 99]], dtype=F)


def _qtable(base, quality):
    scale = 5000.0 / quality if quality < 50 else 200.0 - 2.0 * quality
    return np.clip(np.floor((base * scale + 50.0) / 100.0), 1.0, 255.0).astype(F)


def _h16(m):
    return np.ascontiguousarray(np.asarray(m)).astype(np.float16)


def build_consts():
    QL = _qtable(_LUMA, QUALITY)
    QC = _qtable(_CHROMA, QUALITY)
    k = np.arange(8)
    D = np.sqrt(2.0 / 8.0) * np.cos((2 * k[None, :] + 1) * k[:, None] * np.pi / 16.0)
    D[0, :] /= np.sqrt(2.0)
    D = D.astype(F)
    P = np.zeros((8, 16), F)
    for i in range(8):
        P[i, 2 * i] = 0.5
        P[i, 2 * i + 1] = 0.5
    E = (D @ P).astype(F)          # [8,16] vDCT + 2x downsample
    V = (2.0 * E.T).astype(F)      # [16,8] 2x upsample + IDCT
    I8 = np.eye(8, dtype=F)
    I16 = np.eye(16, dtype=F)
    cY = np.array([0.299, 0.587, 0.114], F)
    cCb = np.array([-0.168736, -0.331264, 0.5], F)
    cCr = np.array([0.5, -0.418688, -0.081312], F)

    c = {}
    ky = np.kron(I16, D.T)         # [128,128] vDCT luma (cols = fh)
    kc = np.kron(I8, E.T)          # [128,64]  vDCT+down chroma
    for ch in range(3):
        mv = np.concatenate(
            [cY[ch] * ky, cCb[ch] * kc, cCr[ch] * kc], axis=1).astype(F)
        c[f"mv1_{ch}"] = _h16(mv)                       # [128,256]
    c["sp2y"] = _h16(ky)                                # [128,128]
    c["sp2c"] = _h16(kc)                                # [128,64] unpadded
    c["mv3y"] = _h16(np.kron(I16, D))                   # [128,128] hIDCT
    c["mv3c"] = _h16(np.kron(I16, V.T))                 # [128,256] hIDCT+up
    s = F(1.0) / F(255.0)
    c["sp4y"] = _h16(s * np.kron(I16, D))               # [128,128] vIDCT/255
    sp4c = np.kron(I16, V).T.astype(F)                  # [128 fhc, 256 h]
    for half in range(2):
        sl = np.ascontiguousarray(sp4c[:, 128 * half:128 * (half + 1)])
        c[f"sp4_h{half}_rcr"] = _h16(F(1.402) * s * sl)
        c[f"sp4_h{half}_gcb"] = _h16(F(-0.344136) * s * sl)
        c[f"sp4_h{half}_gcr"] = _h16(F(-0.714136) * s * sl)
        c[f"sp4_h{half}_bcb"] = _h16(F(1.772) * s * sl)
    # quant tables: luma [128 fw, 8 (2 wchunk x 4 band), 128 fh]
    pp, ff = np.meshgrid(np.arange(128), np.arange(128), indexing="ij")
    tyq = QL[ff % 8, pp % 8].astype(F)                  # [fw, fh] pattern
    tY = np.broadcast_to(tyq[:, None, :], (128, 8, 128))
    c["taby"] = _h16(tY.reshape(128, 1024))
    c["rtaby"] = _h16(1.0 / tY.reshape(128, 1024))
    tcq = QC[ff % 8, pp % 8].astype(F)
    tC = np.broadcast_to(tcq[:, None, :], (128, 2, 128))
    c["tabc"] = _h16(tC.reshape(128, 256))
    c["rtabc"] = _h16(1.0 / tC.reshape(128, 256))
    # pack everything into one [128, 5120] fp16 tensor (single DMA)
    cc = np.concatenate([c[k] for k, _ in CONST_LAYOUT], axis=1)
    assert cc.shape == (128, CC_COLS), cc.shape
    return {"cc": np.ascontiguousarray(cc)}


CONST_LAYOUT = [
    ("mv1_0", 256), ("mv1_1", 256), ("mv1_2", 256),
    ("sp2y", 128), ("sp2c", 64),
    ("mv3y", 128), ("mv3c", 256),
    ("sp4y", 128),
    ("sp4_h0_rcr", 128), ("sp4_h0_gcb", 128),
    ("sp4_h0_gcr", 128), ("sp4_h0_bcb", 128),
    ("sp4_h1_rcr", 128), ("sp4_h1_gcb", 128),
    ("sp4_h1_gcr", 128), ("sp4_h1_bcb", 128),
    ("taby", 1024), ("rtaby", 1024),
    ("tabc", 256), ("rtabc", 256),
]
CC_COLS = sum(n for _, n in CONST_LAYOUT)


def build_nc():
    Alu = mybir.AluOpType
    Act = mybir.ActivationFunctionType
    nc = bacc.Bacc("TRN2", target_bir_lowering=False, debug=False,
                   num_devices=N_CORES)
    x_d = nc.dram_tensor("x", [B_PER_CORE, 512, 512, 3], DT,
                         kind="ExternalInput").ap()
    o_d = nc.dram_tensor("out", [B_PER_CORE, 512, 512, 3], DT,
                         kind="ExternalOutput").ap()
    cc_d = nc.dram_tensor("cc", [128, CC_COLS], F16, kind="ExternalInput").ap()

    with tile.TileContext(nc) as tc:
        with (
            tc.tile_pool(name="cpool", bufs=1) as cpool,
            tc.tile_pool(name="iopool", bufs=8) as iopool,
            tc.tile_pool(name="opool", bufs=4) as opool,
            tc.tile_pool(name="u8pool", bufs=5) as u8pool,
            tc.tile_pool(name="u8fpool", bufs=8) as u8fpool,
            tc.tile_pool(name="m1pool", bufs=5) as m1pool,
            tc.tile_pool(name="kpool", bufs=3) as kpool,
            tc.tile_pool(name="krpool", bufs=3) as krpool,
            tc.tile_pool(name="dqpool", bufs=8) as dqpool,
            tc.tile_pool(name="m3pool", bufs=6) as m3pool,
            # PSUM: 8 banks x 2KB total
            tc.tile_pool(name="p1ps", bufs=2, space="PSUM") as p1ps,    # 2 banks
            tc.tile_pool(name="midps", bufs=2, space="PSUM") as midps,  # 2 banks
            tc.tile_pool(name="cps", bufs=2, space="PSUM") as cps,      # 1 bank
            tc.tile_pool(name="p4ps", bufs=2, space="PSUM") as p4ps,    # 2 banks
        ):
            cc_t = cpool.tile([128, CC_COLS], F16, tag="cc", name="cc")
            # split const load: mv1 (first 768 cols) lands before p1 needs it;
            # the rest streams after image 0's input tiles
            nc.sync.dma_start(out=cc_t[:, 0:768], in_=cc_d[:, 0:768])
            ct = {}
            off = 0
            for k, n in CONST_LAYOUT:
                ap = cc_t[:, off:off + n]
                if k in ("taby", "rtaby"):
                    ap = ap.rearrange("p (a b) -> p a b", a=8)
                elif k in ("tabc", "rtabc"):
                    ap = ap.rearrange("p (a b) -> p a b", a=2)
                ct[k] = ap
                off += n

            S = {}  # per-image state: u8f, deqy, deqc

            def stage_A(b):
                """Load + u8 = floor(255x) = u8convert(255x - 0.5).

                Fill phase (b<=1): spread S1 over ACT/DVE/Pool to shorten
                the pipeline lead-in; steady state: S1a on ACT, S1b on Pool.
                """
                if b <= 1:
                    s1a_eng = ["act", "dve", "act", "dve"]
                    s1b_eng = ["pool", "act", "dve", "pool"]
                else:
                    s1a_eng = ["act"] * 4
                    s1b_eng = ["pool"] * 4
                u8f = []
                for r in range(4):
                    xin = iopool.tile([128, 512, 3], DT, tag="xin", name="xin")
                    nc.sync.dma_start(out=xin[:],
                                      in_=x_d[b, 128 * r:128 * (r + 1)])
                    u8t = u8pool.tile([128, 512, 3], U8, tag="u8", name="u8t")
                    if s1a_eng[r] == "act":
                        nc.scalar.activation(u8t[:], xin[:], Act.Copy,
                                             bias=-0.5, scale=255.0)
                    else:
                        nc.vector.tensor_scalar(
                            out=u8t[:], in0=xin[:], scalar1=255.0, scalar2=0.5,
                            op0=Alu.mult, op1=Alu.subtract)
                    uft = u8fpool.tile([128, 512, 3], F16, tag="u8f", name="uft")
                    if s1b_eng[r] == "pool":
                        nc.gpsimd.tensor_copy(uft[:], u8t[:])
                    elif s1b_eng[r] == "act":
                        nc.scalar.copy(uft[:], u8t[:])
                    else:
                        nc.vector.tensor_copy(uft[:], u8t[:])
                    u8f.append(uft)
                S[b] = {"u8f": u8f}
                if b == 0:
                    nc.sync.dma_start(out=cc_t[:, 768:], in_=cc_d[:, 768:])

            def stage_F(b):
                """p1 (vDCT+color), p2 (hDCT), quantize."""
                u8f = S[b]["u8f"]
                m1 = []
                for jc in range(4):
                    m1t = m1pool.tile([128, 4, 256], F16, tag="m1", name="m1t")
                    for g in range(2):  # r pairs (0,1) and (2,3)
                        psA = p1ps.tile([128, 2, 256], DT, tag="p1", name="psA")
                        for rr in range(2):
                            r = 2 * g + rr
                            for ch in range(3):
                                nc.tensor.matmul(
                                    psA[:, rr, :],
                                    u8f[r][:, 128 * jc:128 * (jc + 1), ch],
                                    ct[f"mv1_{ch}"][:],
                                    start=(ch == 0), stop=(ch == 2))
                        nc.scalar.copy(m1t[:, 2 * g:2 * (g + 1), :], psA[:])
                    m1.append(m1t)

                deqy = []
                for t in range(2):
                    kp = kpool.tile([128, 8, 128], DT, tag="kp", name="kp")
                    for h in range(2):
                        ps2 = midps.tile([128, 4, 128], DT, tag="mid",
                                         name="ps2")
                        nc.tensor.matmul(
                            ps2[:], ct["sp2y"][:],
                            m1[2 * t + h][:, :, 0:128], start=True, stop=True)
                        nc.vector.tensor_tensor(
                            out=kp[:, 4 * h:4 * (h + 1), :], in0=ps2[:],
                            in1=ct["rtaby"][:, 0:4, :], op=Alu.mult)
                    kr = krpool.tile([128, 8, 128], F16, tag="kr", name="kr")
                    nc.gpsimd.tensor_scalar(
                        out=kr[:], in0=kp[:], scalar1=C_RNE, scalar2=C_RNE,
                        op0=Alu.add, op1=Alu.subtract)
                    dq = dqpool.tile([128, 8, 128], F16, tag="dqy", name="dqy")
                    nc.vector.tensor_tensor(
                        out=dq[:], in0=kr[:], in1=ct["taby"][:], op=Alu.mult)
                    deqy.append(dq)

                deqc = {0: [], 1: []}
                for chi, c0 in ((0, 128), (1, 192)):
                    for t_ in range(2):
                        kpc = kpool.tile([128, 2, 128], DT, tag="kpc",
                                         name="kpc")
                        psc = cps.tile([128, 2, 128], DT, tag="psc",
                                       name="psc")
                        for h in range(2):
                            r2 = 2 * t_ + h
                            nc.tensor.matmul(
                                psc[64 * h:64 * (h + 1), :, :], ct["sp2c"][:],
                                m1[r2][:, :, c0:c0 + 64], start=True, stop=True,
                                tile_position=(0, 64 * h))
                        nc.vector.tensor_tensor(
                            out=kpc[:], in0=psc[:], in1=ct["rtabc"][:],
                            op=Alu.mult)
                        krc = krpool.tile([128, 2, 128], F16, tag="krc",
                                          name="krc")
                        nc.gpsimd.tensor_scalar(
                            out=krc[:], in0=kpc[:], scalar1=C_RNE,
                            scalar2=C_RNE, op0=Alu.add, op1=Alu.subtract)
                        dqc = dqpool.tile([128, 2, 128], F16, tag="dqc",
                                          name="dqc")
                        nc.vector.tensor_tensor(
                            out=dqc[:], in0=krc[:], in1=ct["tabc"][:],
                            op=Alu.mult)
                        deqc[chi].append(dqc)
                S[b].update(deqy=deqy, deqc=deqc)

            def p3y_band(b, jc3):
                deqy = S[b]["deqy"]
                ps3 = midps.tile([128, 4, 128], DT, tag="mid", name="ps3")
                for r3 in range(4):
                    nc.tensor.matmul(
                        ps3[:, r3, :],
                        deqy[r3 // 2][:, 4 * (r3 % 2) + jc3, :],
                        ct["mv3y"][:], start=True, stop=True)
                mt = m3pool.tile([128, 4, 128], F16, tag="m3y", name="m3y")
                nc.scalar.copy(mt[:], ps3[:])
                return mt

            def p3c_pair(b, chi, jc3):
                deqc = S[b]["deqc"]
                ps3c = midps.tile([128, 4, 128], DT, tag="mid", name="ps3c")
                for r3 in range(2):
                    nc.tensor.matmul(
                        ps3c[:, 2 * r3:2 * (r3 + 1), :],
                        deqc[chi][r3][:, jc3, :],
                        ct["mv3c"][:], start=True, stop=True)
                mtc = m3pool.tile([128, 4, 128], F16, tag="m3c", name="m3c")
                nc.scalar.copy(mtc[:], ps3c[:])
                return mtc

            def stage_B(b):
                """p3 (hIDCT), p4 (vIDCT+color), clip, store — interleaved
                per band so the first out-DMA starts early."""
                m3y = {}
                m3c = {0: {}, 1: {}}
                for r in range(4):
                    half, rc = r % 2, r // 2
                    if r not in m3y:
                        m3y[r] = p3y_band(b, r)
                    if rc not in m3c[0]:
                        m3c[0][rc] = p3c_pair(b, 0, rc)
                        m3c[1][rc] = p3c_pair(b, 1, rc)
                    if r + 1 < 4:
                        m3y[r + 1] = p3y_band(b, r + 1)
                    my = m3y[r][:]
                    mcb = m3c[0][rc][:]
                    mcr = m3c[1][rc][:]
                    ot = opool.tile([128, 512, 3], DT, tag="ot", name="ot")
                    chans = (
                        (0, ((ct["sp4y"], my), (ct[f"sp4_h{half}_rcr"], mcr))),
                        (1, ((ct["sp4y"], my), (ct[f"sp4_h{half}_gcb"], mcb),
                             (ct[f"sp4_h{half}_gcr"], mcr))),
                        (2, ((ct["sp4y"], my), (ct[f"sp4_h{half}_bcb"], mcb))),
                    )
                    for chn, terms in chans:
                        ps4 = p4ps.tile([128, 512], DT, tag="p4", name="ps4")
                        n = len(terms)
                        for i, (st, mv) in enumerate(terms):
                            nc.tensor.matmul(ps4[:], st[:], mv,
                                             start=(i == 0), stop=(i == n - 1))
                        nc.vector.tensor_scalar(
                            out=ot[:, :, chn], in0=ps4[:], scalar1=1.0,
                            scalar2=0.0, op0=Alu.min, op1=Alu.max)
                    nc.sync.dma_start(out=o_d[b, 128 * r:128 * (r + 1)],
                                      in_=ot[:])
                del S[b]

            # software pipeline: A(b+2) | B(b) | F(b+1)
            B = B_PER_CORE
            stage_A(0)
            stage_A(1)
            stage_F(0)
            for b in range(B):
                if b + 2 < B:
                    stage_A(b + 2)
                stage_B(b)
                if b + 1 < B:
                    stage_F(b + 1)

    nc.compile()
    return nc


_CACHE = {}


def kernel(x: np.ndarray) -> np.ndarray:
    assert x.shape == (32, 512, 512, 3)
    if "nc" not in _CACHE:
        _CACHE["nc"] = build_nc()
        _CACHE["consts"] = build_consts()
    nc = _CACHE["nc"]
    consts = _CACHE["consts"]
    xs = np.ascontiguousarray(x.astype(F))
    in_maps = []
    for i in range(N_CORES):
        m = {"x": xs[B_PER_CORE * i:B_PER_CORE * (i + 1)]}
        m.update(consts)
        in_maps.append(m)
    res = run_bass_kernel_spmd(nc, in_maps, list(range(N_CORES)))
    out = np.concatenate([res.results[i]["out"] for i in range(N_CORES)], axis=0)
    return out.astype(np.float32)
